# revision 34
# baseline (speedup 1.0000x reference)
"""ContentOnlyPhasorBlock on 8 Trainium2 NeuronCores — v2 (all-bf16).

Causal linear attention reformulation (see reference): per core 256 rows,
chunk state S = phi_k^T V AllGathered (bf16), prefix combined on the PE
via per-core 0/1 diagonal weights. LayerNorm is folded algebraically into
the output projection:  y = x + rstd*(r@W) - mu*rstd*(1@W) + b_eff,
so r is consumed in both orientations (r for bn_stats, r^T as matmul lhsT)
without any transposes of the normalized tensor.

All matmul operands are bf16 (1 PE cycle/row at any free size, half the
HBM bytes); PSUM accumulation and the residual path stay fp32.
Host-side precision sim: rel_err ~1.5e-3 vs the 2e-2 gate.
"""
import sys
if '/opt/trn_rl_repo' not in sys.path:
    sys.path.insert(0, '/opt/trn_rl_repo')
import math
import numpy as np
import ml_dtypes
import concourse.bass as bass
import concourse.bacc as bacc
import concourse.mybir as mybir
import concourse.tile as tile
from concourse.bass_utils import run_bass_kernel_spmd

AF = mybir.ActivationFunctionType
ALU = mybir.AluOpType
F32 = mybir.dt.float32
BF = mybir.dt.bfloat16

B, L, D, K = 1, 2048, 512, 64
NCORES = 8
R = L // NCORES          # 256 rows per core
NB = R // 128            # 2 l-blocks
ND = D // 128            # 4 d-tiles

RUN_KWARGS = {}          # test harness can inject trace=True etc.
LAST_RESULTS = None
_PROGRAM_CACHE = {}
DEBUG_DUMP = False
WARMUP_COLLECTIVE = True


def _build_program():
    nc = bacc.Bacc("TRN2", target_bir_lowering=False, debug=False,
                   num_devices=NCORES)

    din = {}
    def inp(name, shape, dt=BF):
        din[name] = nc.dram_tensor(name, list(shape), dt, kind="ExternalInput")
        return din[name]

    wk_d = inp("wk", [D, 768])                  # kw1 | w2k | wam
    wv_d = inp("wv", [D, D])                    # v_w
    wq_d = inp("wq", [D, 640])                  # qw1 | w2q
    wo_d = inp("wo", [D, D])                    # W_eff
    xTp_d = inp("xTp", [128, ND * R])
    cpackf_d = inp("cpackf", [128, 16], F32)
    cpackb_d = inp("cpackb", [128, 1024])       # wdiag x7 | ident
    maskf_d = inp("maskf", [128, 128], F32)
    miscv_d = inp("miscv", [1, D])              # vb
    misc1_d = inp("misc1", [2, D])              # -w1row | b_eff
    x_d = inp("x_rm", [R, D], F32)
    y_d = nc.dram_tensor("y", [R, D], F32, kind="ExternalOutput")
    if DEBUG_DUMP:
        dbg_r_d = nc.dram_tensor("dbg_r", [R, D], F32, kind="ExternalOutput")
        dbg_rw_d = nc.dram_tensor("dbg_rw", [R, D], F32, kind="ExternalOutput")

    with tile.TileContext(nc) as tc:
        with tc.tile_pool(name="sb", bufs=1) as sbp, \
             tc.tile_pool(name="ps", bufs=1, space="PSUM") as ps, \
             tc.tile_pool(name="dr", bufs=1, space="DRAM") as dr:

            # ---------- small SBUF constants ----------
            ones1 = sbp.tile([1, 128], BF, name="ones1")
            nc.gpsimd.memset(ones1[:], 1.0)
            wb = [sbp.tile([2, 128], BF, name=f"wb{lb}") for lb in range(NB)]
            if WARMUP_COLLECTIVE:
                # tiny AllGather at t=0: pre-syncs the cores and pays the
                # collective channel-setup cost during the front DMA/compute
                wu_sb = sbp.tile([1, 16], BF, name="wu_sb")
                nc.gpsimd.memset(wu_sb[:], 0.0)
                wu_in = dr.tile([1, 16], BF, name="wu_in")
                wu_out = dr.tile([NCORES, 1, 16], BF, addr_space="Shared",
                                 name="wu_out")
                nc.sync.dma_start(wu_in[:], wu_sb[:])
                nc.gpsimd.collective_compute(
                    "AllGather", ALU.bypass,
                    replica_groups=[list(range(NCORES))],
                    ins=[wu_in[:]], outs=[wu_out[:]],
                )

            # ---------- front DMA (SP ring, k-path weights first) ----------
            def load_rows(name, dram, p, cols, n):
                ts = []
                for t in range(n):
                    tl = sbp.tile([p, cols], BF, name=f"{name}{t}")
                    nc.sync.dma_start(tl[:], dram[t * p:(t + 1) * p, :])
                    ts.append(tl)
                return ts

            cpackf = sbp.tile([128, 16], F32, name="cpackf")
            nc.sync.dma_start(cpackf[:], cpackf_d[:])
            xTp = sbp.tile([128, ND * R], BF, name="xTp")
            nc.sync.dma_start(xTp[:], xTp_d[:])
            wk = load_rows("wk", wk_d, 128, 768, ND)
            cpackb = sbp.tile([128, 1024], BF, name="cpackb")
            nc.sync.dma_start(cpackb[:], cpackb_d[:])
            wv = load_rows("wv", wv_d, 128, D, ND)
            maskf = sbp.tile([128, 128], F32, name="maskf")
            nc.sync.dma_start(maskf[:], maskf_d[:])
            miscv = sbp.tile([1, D], BF, name="miscv")
            nc.sync.dma_start(miscv[:], miscv_d[:])
            misc1 = sbp.tile([2, D], BF, name="misc1")
            nc.sync.dma_start(misc1[:], misc1_d[:])
            wq = load_rows("wq", wq_d, 128, 640, ND)
            wo = load_rows("wo", wo_d, 128, D, ND)

            sinsc, sinbs = cpackf[:, 0:1], cpackf[:, 1:2]
            epscol = [cpackf[:, 2:3], cpackf[:, 3:4]]
            b1k = [cpackf[:, 4 + i:5 + i] for i in range(ND)]
            b1q = [cpackf[:, 8 + i:9 + i] for i in range(ND)]
            b2k, b2q, bam = cpackf[:, 12:13], cpackf[:, 13:14], cpackf[:, 14:15]
            identb = cpackb[:, 896:1024]

            def xT(dj, lo=0, sz=R):
                return xTp[:, dj * R + lo: dj * R + lo + sz]

            # ---------- k path ----------
            hk = []
            for do in range(ND):
                hps = ps.tile([128, R], F32, name=f"hk_ps{do}", tag="pa", bufs=2)
                for dj in range(ND):
                    nc.tensor.matmul(hps[:], wk[dj][:, do*128:(do+1)*128],
                                     xT(dj), start=(dj == 0), stop=(dj == ND - 1))
                h = sbp.tile([128, R], BF, name=f"hk{do}")
                nc.scalar.activation(h[:], hps[:], AF.Gelu, bias=b1k[do])
                hk.append(h)
            pkps = ps.tile([128, R], F32, name="pk_ps", tag="pa", bufs=2)
            for do in range(ND):
                nc.tensor.matmul(pkps[:], wk[do][:, 512:640], hk[do][:],
                                 start=(do == 0), stop=(do == ND - 1))
            tk = sbp.tile([128, R], F32, name="tk")
            nc.scalar.activation(tk[:], pkps[:], AF.Tanh, bias=b2k)
            nc.scalar.activation(tk[0:64, :], tk[0:64, :], AF.Abs)
            csk = sbp.tile([128, R], F32, name="csk")
            nc.scalar.activation(csk[:], tk[:], AF.Sin, bias=sinbs, scale=sinsc)

            # ---------- amp (softplus via Exp+Ln, one table set) ----------
            amps = ps.tile([128, R], F32, name="am_ps", tag="pa", bufs=2)
            for dj in range(ND):
                nc.tensor.matmul(amps[:], wk[dj][:, 640:768], xT(dj),
                                 start=(dj == 0), stop=(dj == ND - 1))
            e_sb = sbp.tile([128, R], F32, name="e_sb")
            nc.scalar.activation(e_sb[:], amps[:], AF.Exp, bias=bam)
            al_sb = sbp.tile([128, R], F32, name="al_sb")
            nc.scalar.activation(al_sb[:], e_sb[:], AF.Ln, bias=1.0)

            phik = sbp.tile([128, R], BF, name="phik")
            nc.vector.scalar_tensor_tensor(phik[:], al_sb[:], 0.1, csk[:],
                                           ALU.add, ALU.mult)

            # ---------- V ----------
            V = []
            for lb in range(NB):
                vps = ps.tile([128, D], F32, name=f"v_ps{lb}", tag="pv", bufs=3)
                for dj in range(ND):
                    nc.tensor.matmul(vps[:], xT(dj, lb * 128, 128), wv[dj][:],
                                     start=(dj == 0), stop=False)
                nc.tensor.matmul(vps[:], ones1[0:1, :], miscv[:],
                                 start=False, stop=True, skip_group_check=True)
                v = sbp.tile([128, D], BF, name=f"V{lb}")
                nc.vector.tensor_copy(v[:], vps[:])
                V.append(v)

            # ---------- chunk state S + AllGather ----------
            phik_rm = []
            for tb in range(NB):
                trp = ps.tile([128, R], F32, name=f"tr_ps{tb}", tag="pa", bufs=2)
                nc.tensor.matmul(trp[:, 0:128], phik[:, tb*128:(tb+1)*128],
                                 identb, start=True, stop=True)
                k_rm = sbp.tile([128, 128], BF, name=f"phik_rm{tb}")
                nc.vector.tensor_copy(k_rm[:], trp[:, 0:128])
                phik_rm.append(k_rm)
            sps = ps.tile([128, D], F32, name="s_ps", tag="pv", bufs=3)
            for tb in range(NB):
                nc.tensor.matmul(sps[:], phik_rm[tb][:], V[tb][:],
                                 start=(tb == 0), stop=(tb == NB - 1))
            s_sb = sbp.tile([128, D], BF, name="s_sb")
            nc.vector.tensor_copy(s_sb[:], sps[:])
            x_rm = []
            for lb in range(NB):
                xr = sbp.tile([128, D], F32, name=f"x_rm{lb}")
                nc.sync.dma_start(xr[:], x_d[lb*128:(lb+1)*128, :])
                x_rm.append(xr)
            cc_in = dr.tile([128, D], BF, name="cc_in")
            cc_out = dr.tile([NCORES, 128, D], BF, addr_space="Shared",
                             name="cc_out")
            nc.sync.dma_start(cc_in[:], s_sb[:])
            nc.gpsimd.collective_compute(
                "AllGather", ALU.bypass,
                replica_groups=[list(range(NCORES))],
                ins=[cc_in[:]], outs=[cc_out[:]],
            )

            # ---------- q path (fills the AllGather window) ----------
            hq = []
            for do in range(ND):
                hps = ps.tile([128, R], F32, name=f"hq_ps{do}", tag="pa", bufs=2)
                for dj in range(ND):
                    nc.tensor.matmul(hps[:], wq[dj][:, do*128:(do+1)*128],
                                     xT(dj), start=(dj == 0), stop=(dj == ND - 1))
                h = sbp.tile([128, R], BF, name=f"hq{do}")
                nc.scalar.activation(h[:], hps[:], AF.Gelu, bias=b1q[do])
                hq.append(h)
            pqps = ps.tile([128, R], F32, name="pq_ps", tag="pa", bufs=2)
            for do in range(ND):
                nc.tensor.matmul(pqps[:], wq[do][:, 512:640], hq[do][:],
                                 start=(do == 0), stop=(do == ND - 1))
            tq = sbp.tile([128, R], F32, name="tq")
            nc.scalar.activation(tq[:], pqps[:], AF.Tanh, bias=b2q)
            nc.scalar.activation(tq[0:64, :], tq[0:64, :], AF.Abs)
            csq = sbp.tile([128, R], F32, name="csq")
            nc.scalar.activation(csq[:], tq[:], AF.Sin, bias=sinbs, scale=sinsc)
            phiq = sbp.tile([128, R], BF, name="phiq")
            nc.vector.scalar_tensor_tensor(phiq[:], al_sb[:], 0.1, csq[:],
                                           ALU.add, ALU.mult)

            # ---------- intra-chunk scores ----------
            a0ps = ps.tile([128, R], F32, name="a0_ps", tag="pa", bufs=2)
            nc.tensor.matmul(a0ps[:], phik[:, 0:128], phiq[:], start=True, stop=True)
            a0 = sbp.tile([128, R], BF, name="a0")
            nc.vector.tensor_tensor(a0[:, 0:128], a0ps[:, 0:128], maskf[:], ALU.mult)
            nc.vector.tensor_copy(a0[:, 128:256], a0ps[:, 128:256])
            a1ps = ps.tile([128, R], F32, name="a1_ps", tag="pa", bufs=2)
            nc.tensor.matmul(a1ps[:, 0:128], phik[:, 128:256], phiq[:, 128:256],
                             start=True, stop=True)
            a1 = sbp.tile([128, 128], BF, name="a1")
            nc.vector.tensor_tensor(a1[:], a1ps[:, 0:128], maskf[:], ALU.mult)

            # ---------- r (l,d) and rT (d,l) intra parts ----------
            rps = []
            for lb in range(NB):
                rp = ps.tile([128, D], F32, name=f"r_ps{lb}", tag="pv", bufs=3)
                if lb == 0:
                    nc.tensor.matmul(rp[:], a0[:, 0:128], V[0][:],
                                     start=True, stop=False)
                else:
                    nc.tensor.matmul(rp[:], a0[:, 128:256], V[0][:],
                                     start=True, stop=False)
                    nc.tensor.matmul(rp[:], a1[:], V[1][:],
                                     start=False, stop=False)
                rps.append(rp)
            # One open accumulation group per PSUM bank at a time: each pair
            # tile runs dt-even's full group (intra now, inter later) before
            # dt-odd's group opens post-collective.
            rtps = []
            for pair in range(2):            # pair p holds dtiles 2p, 2p+1
                rt = ps.tile([128, D], F32, name=f"rt_ps{pair}", tag="prt", bufs=2)
                dt = pair * 2
                nc.tensor.matmul(rt[:, 0:256],
                                 V[0][:, dt*128:(dt+1)*128], a0[:],
                                 start=True, stop=False,
                                 skip_group_check=True)
                nc.tensor.matmul(rt[:, 128:256],
                                 V[1][:, dt*128:(dt+1)*128], a1[:],
                                 start=False, stop=False,
                                 skip_group_check=True)
                rtps.append(rt)

            # ---------- prefix state P from gathered S ----------
            pps = ps.tile([128, D], F32, name="p_ps", tag="pv", bufs=3)
            for j in range(NCORES - 1):
                sa = sbp.tile([128, D], BF, name=f"s_all{j}")
                nc.sync.dma_start(sa[:], cc_out[j])
                nc.tensor.matmul(pps[:], cpackb[:, j*128:(j+1)*128], sa[:],
                                 start=(j == 0), stop=(j == NCORES - 2))
            p_sb = sbp.tile([128, D], BF, name="p_sb")
            nc.scalar.copy(p_sb[:], pps[:])

            # ---------- inter-chunk terms ----------
            for lb in range(NB):
                nc.tensor.matmul(rps[lb][:], phiq[:, lb*128:(lb+1)*128], p_sb[:],
                                 start=False, stop=True, skip_group_check=True)
            for pair in range(2):
                dt0, dt1 = pair * 2, pair * 2 + 1
                # close dt-even's group (inter term), then dt-odd's full group
                nc.tensor.matmul(rtps[pair][:, 0:256],
                                 p_sb[:, dt0*128:(dt0+1)*128], phiq[:],
                                 start=False, stop=True, skip_group_check=True)
                nc.tensor.matmul(rtps[pair][:, 256:512],
                                 V[0][:, dt1*128:(dt1+1)*128], a0[:],
                                 start=True, stop=False, skip_group_check=True)
                nc.tensor.matmul(rtps[pair][:, 384:512],
                                 V[1][:, dt1*128:(dt1+1)*128], a1[:],
                                 start=False, stop=False, skip_group_check=True)
                nc.tensor.matmul(rtps[pair][:, 256:512],
                                 p_sb[:, dt1*128:(dt1+1)*128], phiq[:],
                                 start=False, stop=True, skip_group_check=True)

            # ---------- LN stats + folded output projection ----------
            rt_sb = []
            for pair in range(2):
                rs = sbp.tile([128, D], BF, name=f"rt_sb{pair}")
                nc.scalar.copy(rs[:], rtps[pair][:])
                rt_sb.append(rs)
            if DEBUG_DUMP:
                for lb in range(NB):
                    rf = sbp.tile([128, D], F32, name=f"dbg_r{lb}")
                    nc.scalar.copy(rf[:], rps[lb][:])
                    nc.sync.dma_start(dbg_r_d[lb*128:(lb+1)*128, :], rf[:])
            rstds = []
            for lb in range(NB):
                bn6 = sbp.tile([128, 6], F32, name=f"bn6_{lb}")
                nc.vector.bn_stats(bn6[:], rps[lb][:])
                bn2 = sbp.tile([128, 2], F32, name=f"bn2_{lb}")
                nc.vector.bn_aggr(bn2[:], bn6[:])
                # irstd = sqrt(var+eps), rstd = 1/irstd (DVE reciprocal)
                irstd = sbp.tile([128, 1], F32, name=f"irstd{lb}")
                nc.scalar.activation(irstd[:], bn2[:, 1:2], AF.Sqrt,
                                     bias=epscol[lb])
                rstd = sbp.tile([128, 1], F32, name=f"rstd{lb}")
                nc.vector.reciprocal(rstd[:], irstd[:])
                rstds.append(rstd)
                # rank-1 coefficients: row0 = mu, row1 = irstd (the final
                # y = rstd*rw + x multiply folds them to -mu*rstd*w1row + b_eff)
                muir = sbp.tile([128, 2], BF, name=f"muir{lb}")
                nc.vector.tensor_copy(muir[:, 0:1], bn2[:, 0:1])
                nc.vector.tensor_copy(muir[:, 1:2], irstd[:])
                btr = ps.tile([2, 128], F32, name=f"btr{lb}", tag="pb", bufs=1)
                nc.tensor.matmul(btr[:], muir[:], identb, start=True, stop=True)
                nc.vector.tensor_copy(wb[lb][:], btr[:])

            for lb in range(NB):
                rw = ps.tile([128, D], F32, name=f"rw_ps{lb}", tag="pv", bufs=3)
                for dt in range(ND):
                    pair, half = dt // 2, dt % 2
                    nc.tensor.matmul(rw[:],
                                     rt_sb[pair][:, half*256 + lb*128: half*256 + (lb+1)*128],
                                     wo[dt][:],
                                     start=(dt == 0), stop=False)
                nc.tensor.matmul(rw[:], wb[lb][:], misc1[:],
                                 start=False, stop=True, skip_group_check=True)
                if DEBUG_DUMP:
                    rwf = sbp.tile([128, D], F32, name=f"dbg_rw{lb}")
                    nc.scalar.copy(rwf[:], rw[:])
                    nc.sync.dma_start(dbg_rw_d[lb*128:(lb+1)*128, :], rwf[:])
                y_sb = sbp.tile([128, D], F32, name=f"y{lb}")
                nc.vector.scalar_tensor_tensor(y_sb[:], rw[:], rstds[lb][:],
                                               x_rm[lb][:], ALU.mult, ALU.add)
                nc.sync.dma_start(y_d[lb*128:(lb+1)*128, :], y_sb[:])

    nc.compile()
    return nc


def _bf16(a):
    return np.ascontiguousarray(np.asarray(a, np.float32).astype(ml_dtypes.bfloat16))


def kernel(**inputs):
    global LAST_RESULTS
    if 'prog' not in _PROGRAM_CACHE:
        _PROGRAM_CACHE['prog'] = _build_program()
    nc = _PROGRAM_CACHE['prog']

    f = {k: np.asarray(v, np.float32) for k, v in inputs.items()}
    x = f['x'][0]                                   # (L, D)
    W_eff = f['ln_g'][:, None] * f['out_w']
    b_eff = f['ln_b'] @ f['out_w'] + f['out_b']
    w1row = W_eff.sum(axis=0)

    wkp = np.concatenate([f['ke_w1'],
                          np.concatenate([f['ke_w2'], f['ke_w2']], 1),
                          np.concatenate([f['amp_w'], f['amp_w']], 1)], 1)
    wqp = np.concatenate([f['qe_w1'],
                          np.concatenate([f['qe_w2'], f['qe_w2']], 1)], 1)

    miscv = f['v_b'][None, :]
    misc1 = np.stack([-w1row, b_eff])

    maskf = (np.arange(128)[None, :] >= np.arange(128)[:, None]).astype(np.float32)

    shared = {
        "wk": _bf16(wkp),
        "wv": _bf16(f['v_w']),
        "wq": _bf16(wqp),
        "wo": _bf16(W_eff),
        "miscv": _bf16(miscv),
        "misc1": _bf16(misc1),
        "maskf": maskf,
    }

    in_maps = []
    for c in range(NCORES):
        xc = x[R*c:R*(c+1)]
        xTp = np.zeros((128, ND * R), np.float32)
        for t in range(ND):
            xTp[:, t*R:(t+1)*R] = xc[:, t*128:(t+1)*128].T
        cpackf = np.zeros((128, 16), np.float32)
        cpackf[0:64, 0] = -math.pi
        cpackf[64:128, 0] = math.pi
        cpackf[0:64, 1] = math.pi / 2
        for lb in range(NB):
            gl = np.arange(c*R + lb*128, c*R + (lb+1)*128, dtype=np.float64)
            cpackf[:, 2 + lb] = (1e-5 * K * (gl + 1)).astype(np.float32)
        for i in range(ND):
            cpackf[:, 4 + i] = f['ke_b1'][i*128:(i+1)*128]
            cpackf[:, 8 + i] = f['qe_b1'][i*128:(i+1)*128]
        cpackf[:, 12] = np.concatenate([f['ke_b2'], f['ke_b2']])
        cpackf[:, 13] = np.concatenate([f['qe_b2'], f['qe_b2']])
        cpackf[:, 14] = np.concatenate([f['amp_b'], f['amp_b']])
        cpackb = np.zeros((128, 1024), np.float32)
        eye = np.eye(128, dtype=np.float32)
        for j in range(NCORES - 1):
            if j < c:
                cpackb[:, j*128:(j+1)*128] = eye
        cpackb[:, 896:1024] = eye
        in_maps.append({
            **shared,
            "xTp": _bf16(xTp),
            "cpackf": np.ascontiguousarray(cpackf),
            "cpackb": _bf16(cpackb),
            "x_rm": np.ascontiguousarray(xc),
        })

    res = run_bass_kernel_spmd(nc, in_maps, core_ids=list(range(NCORES)),
                               **RUN_KWARGS)
    LAST_RESULTS = res
    y = np.concatenate([res.results[c]['y'] for c in range(NCORES)], axis=0)
    return y[None].astype(np.float32)


# revision 35
# speedup vs baseline: 42.8561x; 42.8561x over previous
"""ContentOnlyPhasorBlock on 8 Trainium2 NeuronCores — v2 (all-bf16).

Causal linear attention reformulation (see reference): per core 256 rows,
chunk state S = phi_k^T V AllGathered (bf16), prefix combined on the PE
via per-core 0/1 diagonal weights. LayerNorm is folded algebraically into
the output projection:  y = x + rstd*(r@W) - mu*rstd*(1@W) + b_eff,
so r is consumed in both orientations (r for bn_stats, r^T as matmul lhsT)
without any transposes of the normalized tensor.

All matmul operands are bf16 (1 PE cycle/row at any free size, half the
HBM bytes); PSUM accumulation and the residual path stay fp32.
Host-side precision sim: rel_err ~1.5e-3 vs the 2e-2 gate.
"""
import sys
if '/opt/trn_rl_repo' not in sys.path:
    sys.path.insert(0, '/opt/trn_rl_repo')
import math
import numpy as np
import ml_dtypes
import concourse.bass as bass
import concourse.bacc as bacc
import concourse.mybir as mybir
import concourse.tile as tile
from concourse.bass_utils import run_bass_kernel_spmd

AF = mybir.ActivationFunctionType
ALU = mybir.AluOpType
F32 = mybir.dt.float32
BF = mybir.dt.bfloat16

B, L, D, K = 1, 2048, 512, 64
NCORES = 8
R = L // NCORES          # 256 rows per core
NB = R // 128            # 2 l-blocks
ND = D // 128            # 4 d-tiles

RUN_KWARGS = {}          # test harness can inject trace=True etc.
LAST_RESULTS = None
_PROGRAM_CACHE = {}
DEBUG_DUMP = False
WARMUP_COLLECTIVE = False


def _build_program():
    nc = bacc.Bacc("TRN2", target_bir_lowering=False, debug=False,
                   num_devices=NCORES)

    din = {}
    def inp(name, shape, dt=BF):
        din[name] = nc.dram_tensor(name, list(shape), dt, kind="ExternalInput")
        return din[name]

    wk_d = inp("wk", [D, 768])                  # kw1 | w2k | wam
    wv_d = inp("wv", [D, D])                    # v_w
    wq_d = inp("wq", [D, 640])                  # qw1 | w2q
    wo_d = inp("wo", [D, D])                    # W_eff
    xTp_d = inp("xTp", [128, ND * R])
    cpackf_d = inp("cpackf", [128, 16], F32)
    cpackb_d = inp("cpackb", [128, 1024])       # wdiag x7 | ident
    maskf_d = inp("maskf", [128, 128], F32)
    miscv_d = inp("miscv", [1, D])              # vb
    misc1_d = inp("misc1", [2, D])              # -w1row | b_eff
    x_d = inp("x_rm", [R, D], F32)
    y_d = nc.dram_tensor("y", [R, D], F32, kind="ExternalOutput")
    if DEBUG_DUMP:
        dbg_r_d = nc.dram_tensor("dbg_r", [R, D], F32, kind="ExternalOutput")
        dbg_rw_d = nc.dram_tensor("dbg_rw", [R, D], F32, kind="ExternalOutput")

    with tile.TileContext(nc) as tc:
        with tc.tile_pool(name="sb", bufs=1) as sbp, \
             tc.tile_pool(name="ps", bufs=1, space="PSUM") as ps, \
             tc.tile_pool(name="dr", bufs=1, space="DRAM") as dr:

            # ---------- small SBUF constants ----------
            ones1 = sbp.tile([1, 128], BF, name="ones1")
            nc.gpsimd.memset(ones1[:], 1.0)
            wb = [sbp.tile([2, 128], BF, name=f"wb{lb}") for lb in range(NB)]
            if WARMUP_COLLECTIVE:
                # tiny AllGather at t=0: pre-syncs the cores and pays the
                # collective channel-setup cost during the front DMA/compute
                wu_sb = sbp.tile([1, 16], BF, name="wu_sb")
                nc.gpsimd.memset(wu_sb[:], 0.0)
                wu_in = dr.tile([1, 16], BF, name="wu_in")
                wu_out = dr.tile([NCORES, 1, 16], BF, addr_space="Shared",
                                 name="wu_out")
                nc.sync.dma_start(wu_in[:], wu_sb[:])
                nc.gpsimd.collective_compute(
                    "AllGather", ALU.bypass,
                    replica_groups=[list(range(NCORES))],
                    ins=[wu_in[:]], outs=[wu_out[:]],
                )

            # ---------- front DMA (SP ring, k-path weights first) ----------
            def load_rows(name, dram, p, cols, n):
                ts = []
                for t in range(n):
                    tl = sbp.tile([p, cols], BF, name=f"{name}{t}")
                    nc.sync.dma_start(tl[:], dram[t * p:(t + 1) * p, :])
                    ts.append(tl)
                return ts

            cpackf = sbp.tile([128, 16], F32, name="cpackf")
            nc.sync.dma_start(cpackf[:], cpackf_d[:])
            xTp = sbp.tile([128, ND * R], BF, name="xTp")
            nc.sync.dma_start(xTp[:], xTp_d[:])
            wk = load_rows("wk", wk_d, 128, 768, ND)
            cpackb = sbp.tile([128, 1024], BF, name="cpackb")
            nc.sync.dma_start(cpackb[:], cpackb_d[:])
            wv = load_rows("wv", wv_d, 128, D, ND)
            maskf = sbp.tile([128, 128], F32, name="maskf")
            nc.sync.dma_start(maskf[:], maskf_d[:])
            miscv = sbp.tile([1, D], BF, name="miscv")
            nc.sync.dma_start(miscv[:], miscv_d[:])
            misc1 = sbp.tile([2, D], BF, name="misc1")
            nc.sync.dma_start(misc1[:], misc1_d[:])
            wq = load_rows("wq", wq_d, 128, 640, ND)
            wo = load_rows("wo", wo_d, 128, D, ND)

            sinsc, sinbs = cpackf[:, 0:1], cpackf[:, 1:2]
            epscol = [cpackf[:, 2:3], cpackf[:, 3:4]]
            b1k = [cpackf[:, 4 + i:5 + i] for i in range(ND)]
            b1q = [cpackf[:, 8 + i:9 + i] for i in range(ND)]
            b2k, b2q, bam = cpackf[:, 12:13], cpackf[:, 13:14], cpackf[:, 14:15]
            identb = cpackb[:, 896:1024]

            def xT(dj, lo=0, sz=R):
                return xTp[:, dj * R + lo: dj * R + lo + sz]

            # ---------- k path ----------
            hk = []
            for do in range(ND):
                hps = ps.tile([128, R], F32, name=f"hk_ps{do}", tag="pa", bufs=2)
                for dj in range(ND):
                    nc.tensor.matmul(hps[:], wk[dj][:, do*128:(do+1)*128],
                                     xT(dj), start=(dj == 0), stop=(dj == ND - 1))
                h = sbp.tile([128, R], BF, name=f"hk{do}")
                nc.scalar.activation(h[:], hps[:], AF.Gelu, bias=b1k[do])
                hk.append(h)
            pkps = ps.tile([128, R], F32, name="pk_ps", tag="pa", bufs=2)
            for do in range(ND):
                nc.tensor.matmul(pkps[:], wk[do][:, 512:640], hk[do][:],
                                 start=(do == 0), stop=(do == ND - 1))
            tk = sbp.tile([128, R], F32, name="tk")
            nc.scalar.activation(tk[:], pkps[:], AF.Tanh, bias=b2k)
            nc.scalar.activation(tk[0:64, :], tk[0:64, :], AF.Abs)
            csk = sbp.tile([128, R], F32, name="csk")
            nc.scalar.activation(csk[:], tk[:], AF.Sin, bias=sinbs, scale=sinsc)

            # ---------- amp (softplus via Exp+Ln, one table set) ----------
            amps = ps.tile([128, R], F32, name="am_ps", tag="pa", bufs=2)
            for dj in range(ND):
                nc.tensor.matmul(amps[:], wk[dj][:, 640:768], xT(dj),
                                 start=(dj == 0), stop=(dj == ND - 1))
            e_sb = sbp.tile([128, R], F32, name="e_sb")
            nc.scalar.activation(e_sb[:], amps[:], AF.Exp, bias=bam)
            al_sb = sbp.tile([128, R], F32, name="al_sb")
            nc.scalar.activation(al_sb[:], e_sb[:], AF.Ln, bias=1.0)

            phik = sbp.tile([128, R], BF, name="phik")
            nc.vector.scalar_tensor_tensor(phik[:], al_sb[:], 0.1, csk[:],
                                           ALU.add, ALU.mult)

            # ---------- V ----------
            V = []
            for lb in range(NB):
                vps = ps.tile([128, D], F32, name=f"v_ps{lb}", tag="pv", bufs=3)
                for dj in range(ND):
                    nc.tensor.matmul(vps[:], xT(dj, lb * 128, 128), wv[dj][:],
                                     start=(dj == 0), stop=False)
                nc.tensor.matmul(vps[:], ones1[0:1, :], miscv[:],
                                 start=False, stop=True, skip_group_check=True)
                v = sbp.tile([128, D], BF, name=f"V{lb}")
                nc.vector.tensor_copy(v[:], vps[:])
                V.append(v)

            # ---------- chunk state S + AllGather ----------
            phik_rm = []
            for tb in range(NB):
                trp = ps.tile([128, R], F32, name=f"tr_ps{tb}", tag="pa", bufs=2)
                nc.tensor.matmul(trp[:, 0:128], phik[:, tb*128:(tb+1)*128],
                                 identb, start=True, stop=True)
                k_rm = sbp.tile([128, 128], BF, name=f"phik_rm{tb}")
                nc.vector.tensor_copy(k_rm[:], trp[:, 0:128])
                phik_rm.append(k_rm)
            sps = ps.tile([128, D], F32, name="s_ps", tag="pv", bufs=3)
            for tb in range(NB):
                nc.tensor.matmul(sps[:], phik_rm[tb][:], V[tb][:],
                                 start=(tb == 0), stop=(tb == NB - 1))
            s_sb = sbp.tile([128, D], BF, name="s_sb")
            nc.vector.tensor_copy(s_sb[:], sps[:])
            x_rm = []
            for lb in range(NB):
                xr = sbp.tile([128, D], F32, name=f"x_rm{lb}")
                nc.sync.dma_start(xr[:], x_d[lb*128:(lb+1)*128, :])
                x_rm.append(xr)
            cc_in = dr.tile([128, D], BF, name="cc_in")
            cc_out = dr.tile([NCORES, 128, D], BF, addr_space="Shared",
                             name="cc_out")
            nc.sync.dma_start(cc_in[:], s_sb[:])
            nc.gpsimd.collective_compute(
                "AllGather", ALU.bypass,
                replica_groups=[list(range(NCORES))],
                ins=[cc_in[:]], outs=[cc_out[:]],
            )

            # ---------- q path (fills the AllGather window) ----------
            hq = []
            for do in range(ND):
                hps = ps.tile([128, R], F32, name=f"hq_ps{do}", tag="pa", bufs=2)
                for dj in range(ND):
                    nc.tensor.matmul(hps[:], wq[dj][:, do*128:(do+1)*128],
                                     xT(dj), start=(dj == 0), stop=(dj == ND - 1))
                h = sbp.tile([128, R], BF, name=f"hq{do}")
                nc.scalar.activation(h[:], hps[:], AF.Gelu, bias=b1q[do])
                hq.append(h)
            pqps = ps.tile([128, R], F32, name="pq_ps", tag="pa", bufs=2)
            for do in range(ND):
                nc.tensor.matmul(pqps[:], wq[do][:, 512:640], hq[do][:],
                                 start=(do == 0), stop=(do == ND - 1))
            tq = sbp.tile([128, R], F32, name="tq")
            nc.scalar.activation(tq[:], pqps[:], AF.Tanh, bias=b2q)
            nc.scalar.activation(tq[0:64, :], tq[0:64, :], AF.Abs)
            csq = sbp.tile([128, R], F32, name="csq")
            nc.scalar.activation(csq[:], tq[:], AF.Sin, bias=sinbs, scale=sinsc)
            phiq = sbp.tile([128, R], BF, name="phiq")
            nc.vector.scalar_tensor_tensor(phiq[:], al_sb[:], 0.1, csq[:],
                                           ALU.add, ALU.mult)

            # ---------- intra-chunk scores ----------
            a0ps = ps.tile([128, R], F32, name="a0_ps", tag="pa", bufs=2)
            nc.tensor.matmul(a0ps[:], phik[:, 0:128], phiq[:], start=True, stop=True)
            a0 = sbp.tile([128, R], BF, name="a0")
            nc.vector.tensor_tensor(a0[:, 0:128], a0ps[:, 0:128], maskf[:], ALU.mult)
            nc.vector.tensor_copy(a0[:, 128:256], a0ps[:, 128:256])
            a1ps = ps.tile([128, R], F32, name="a1_ps", tag="pa", bufs=2)
            nc.tensor.matmul(a1ps[:, 0:128], phik[:, 128:256], phiq[:, 128:256],
                             start=True, stop=True)
            a1 = sbp.tile([128, 128], BF, name="a1")
            nc.vector.tensor_tensor(a1[:], a1ps[:, 0:128], maskf[:], ALU.mult)

            # ---------- r (l,d) and rT (d,l) intra parts ----------
            rps = []
            for lb in range(NB):
                rp = ps.tile([128, D], F32, name=f"r_ps{lb}", tag="pv", bufs=3)
                if lb == 0:
                    nc.tensor.matmul(rp[:], a0[:, 0:128], V[0][:],
                                     start=True, stop=False)
                else:
                    nc.tensor.matmul(rp[:], a0[:, 128:256], V[0][:],
                                     start=True, stop=False)
                    nc.tensor.matmul(rp[:], a1[:], V[1][:],
                                     start=False, stop=False)
                rps.append(rp)
            # One open accumulation group per PSUM bank at a time: each pair
            # tile runs dt-even's full group (intra now, inter later) before
            # dt-odd's group opens post-collective.
            rtps = []
            for pair in range(2):            # pair p holds dtiles 2p, 2p+1
                rt = ps.tile([128, D], F32, name=f"rt_ps{pair}", tag="prt", bufs=2)
                dt = pair * 2
                nc.tensor.matmul(rt[:, 0:256],
                                 V[0][:, dt*128:(dt+1)*128], a0[:],
                                 start=True, stop=False,
                                 skip_group_check=True)
                nc.tensor.matmul(rt[:, 128:256],
                                 V[1][:, dt*128:(dt+1)*128], a1[:],
                                 start=False, stop=False,
                                 skip_group_check=True)
                rtps.append(rt)

            # ---------- prefix state P from gathered S ----------
            pps = ps.tile([128, D], F32, name="p_ps", tag="pv", bufs=3)
            for j in range(NCORES - 1):
                sa = sbp.tile([128, D], BF, name=f"s_all{j}")
                nc.sync.dma_start(sa[:], cc_out[j])
                nc.tensor.matmul(pps[:], cpackb[:, j*128:(j+1)*128], sa[:],
                                 start=(j == 0), stop=(j == NCORES - 2))
            p_sb = sbp.tile([128, D], BF, name="p_sb")
            nc.scalar.copy(p_sb[:], pps[:])

            # ---------- inter-chunk terms ----------
            for lb in range(NB):
                nc.tensor.matmul(rps[lb][:], phiq[:, lb*128:(lb+1)*128], p_sb[:],
                                 start=False, stop=True, skip_group_check=True)
            for pair in range(2):
                dt0, dt1 = pair * 2, pair * 2 + 1
                # close dt-even's group (inter term), then dt-odd's full group
                nc.tensor.matmul(rtps[pair][:, 0:256],
                                 p_sb[:, dt0*128:(dt0+1)*128], phiq[:],
                                 start=False, stop=True, skip_group_check=True)
                nc.tensor.matmul(rtps[pair][:, 256:512],
                                 V[0][:, dt1*128:(dt1+1)*128], a0[:],
                                 start=True, stop=False, skip_group_check=True)
                nc.tensor.matmul(rtps[pair][:, 384:512],
                                 V[1][:, dt1*128:(dt1+1)*128], a1[:],
                                 start=False, stop=False, skip_group_check=True)
                nc.tensor.matmul(rtps[pair][:, 256:512],
                                 p_sb[:, dt1*128:(dt1+1)*128], phiq[:],
                                 start=False, stop=True, skip_group_check=True)

            # ---------- LN stats + folded output projection ----------
            rt_sb = []
            for pair in range(2):
                rs = sbp.tile([128, D], BF, name=f"rt_sb{pair}")
                nc.scalar.copy(rs[:], rtps[pair][:])
                rt_sb.append(rs)
            if DEBUG_DUMP:
                for lb in range(NB):
                    rf = sbp.tile([128, D], F32, name=f"dbg_r{lb}")
                    nc.scalar.copy(rf[:], rps[lb][:])
                    nc.sync.dma_start(dbg_r_d[lb*128:(lb+1)*128, :], rf[:])
            rstds = []
            for lb in range(NB):
                bn6 = sbp.tile([128, 6], F32, name=f"bn6_{lb}")
                nc.vector.bn_stats(bn6[:], rps[lb][:])
                bn2 = sbp.tile([128, 2], F32, name=f"bn2_{lb}")
                nc.vector.bn_aggr(bn2[:], bn6[:])
                # irstd = sqrt(var+eps), rstd = 1/irstd (DVE reciprocal)
                irstd = sbp.tile([128, 1], F32, name=f"irstd{lb}")
                nc.scalar.activation(irstd[:], bn2[:, 1:2], AF.Sqrt,
                                     bias=epscol[lb])
                rstd = sbp.tile([128, 1], F32, name=f"rstd{lb}")
                nc.vector.reciprocal(rstd[:], irstd[:])
                rstds.append(rstd)
                # rank-1 coefficients: row0 = mu, row1 = irstd (the final
                # y = rstd*rw + x multiply folds them to -mu*rstd*w1row + b_eff)
                muir = sbp.tile([128, 2], BF, name=f"muir{lb}")
                nc.vector.tensor_copy(muir[:, 0:1], bn2[:, 0:1])
                nc.vector.tensor_copy(muir[:, 1:2], irstd[:])
                btr = ps.tile([2, 128], F32, name=f"btr{lb}", tag="pb", bufs=1)
                nc.tensor.matmul(btr[:], muir[:], identb, start=True, stop=True)
                nc.vector.tensor_copy(wb[lb][:], btr[:])

            for lb in range(NB):
                rw = ps.tile([128, D], F32, name=f"rw_ps{lb}", tag="pv", bufs=3)
                for dt in range(ND):
                    pair, half = dt // 2, dt % 2
                    nc.tensor.matmul(rw[:],
                                     rt_sb[pair][:, half*256 + lb*128: half*256 + (lb+1)*128],
                                     wo[dt][:],
                                     start=(dt == 0), stop=False)
                nc.tensor.matmul(rw[:], wb[lb][:], misc1[:],
                                 start=False, stop=True, skip_group_check=True)
                if DEBUG_DUMP:
                    rwf = sbp.tile([128, D], F32, name=f"dbg_rw{lb}")
                    nc.scalar.copy(rwf[:], rw[:])
                    nc.sync.dma_start(dbg_rw_d[lb*128:(lb+1)*128, :], rwf[:])
                y_sb = sbp.tile([128, D], F32, name=f"y{lb}")
                nc.vector.scalar_tensor_tensor(y_sb[:], rw[:], rstds[lb][:],
                                               x_rm[lb][:], ALU.mult, ALU.add)
                nc.sync.dma_start(y_d[lb*128:(lb+1)*128, :], y_sb[:])

    nc.compile()
    return nc


def _bf16(a):
    return np.ascontiguousarray(np.asarray(a, np.float32).astype(ml_dtypes.bfloat16))


def kernel(**inputs):
    global LAST_RESULTS
    if 'prog' not in _PROGRAM_CACHE:
        _PROGRAM_CACHE['prog'] = _build_program()
    nc = _PROGRAM_CACHE['prog']

    f = {k: np.asarray(v, np.float32) for k, v in inputs.items()}
    x = f['x'][0]                                   # (L, D)
    W_eff = f['ln_g'][:, None] * f['out_w']
    b_eff = f['ln_b'] @ f['out_w'] + f['out_b']
    w1row = W_eff.sum(axis=0)

    wkp = np.concatenate([f['ke_w1'],
                          np.concatenate([f['ke_w2'], f['ke_w2']], 1),
                          np.concatenate([f['amp_w'], f['amp_w']], 1)], 1)
    wqp = np.concatenate([f['qe_w1'],
                          np.concatenate([f['qe_w2'], f['qe_w2']], 1)], 1)

    miscv = f['v_b'][None, :]
    misc1 = np.stack([-w1row, b_eff])

    maskf = (np.arange(128)[None, :] >= np.arange(128)[:, None]).astype(np.float32)

    shared = {
        "wk": _bf16(wkp),
        "wv": _bf16(f['v_w']),
        "wq": _bf16(wqp),
        "wo": _bf16(W_eff),
        "miscv": _bf16(miscv),
        "misc1": _bf16(misc1),
        "maskf": maskf,
    }

    in_maps = []
    for c in range(NCORES):
        xc = x[R*c:R*(c+1)]
        xTp = np.zeros((128, ND * R), np.float32)
        for t in range(ND):
            xTp[:, t*R:(t+1)*R] = xc[:, t*128:(t+1)*128].T
        cpackf = np.zeros((128, 16), np.float32)
        cpackf[0:64, 0] = -math.pi
        cpackf[64:128, 0] = math.pi
        cpackf[0:64, 1] = math.pi / 2
        for lb in range(NB):
            gl = np.arange(c*R + lb*128, c*R + (lb+1)*128, dtype=np.float64)
            cpackf[:, 2 + lb] = (1e-5 * K * (gl + 1)).astype(np.float32)
        for i in range(ND):
            cpackf[:, 4 + i] = f['ke_b1'][i*128:(i+1)*128]
            cpackf[:, 8 + i] = f['qe_b1'][i*128:(i+1)*128]
        cpackf[:, 12] = np.concatenate([f['ke_b2'], f['ke_b2']])
        cpackf[:, 13] = np.concatenate([f['qe_b2'], f['qe_b2']])
        cpackf[:, 14] = np.concatenate([f['amp_b'], f['amp_b']])
        cpackb = np.zeros((128, 1024), np.float32)
        eye = np.eye(128, dtype=np.float32)
        for j in range(NCORES - 1):
            if j < c:
                cpackb[:, j*128:(j+1)*128] = eye
        cpackb[:, 896:1024] = eye
        in_maps.append({
            **shared,
            "xTp": _bf16(xTp),
            "cpackf": np.ascontiguousarray(cpackf),
            "cpackb": _bf16(cpackb),
            "x_rm": np.ascontiguousarray(xc),
        })

    res = run_bass_kernel_spmd(nc, in_maps, core_ids=list(range(NCORES)),
                               **RUN_KWARGS)
    LAST_RESULTS = res
    y = np.concatenate([res.results[c]['y'] for c in range(NCORES)], axis=0)
    return y[None].astype(np.float32)


# revision 45
# speedup vs baseline: 48.8520x; 1.1399x over previous
"""ContentOnlyPhasorBlock on 8 Trainium2 NeuronCores — v2 (all-bf16).

Causal linear attention reformulation (see reference): per core 256 rows,
chunk state S = phi_k^T V AllGathered (bf16), prefix combined on the PE
via per-core 0/1 diagonal weights. LayerNorm is folded algebraically into
the output projection:  y = x + rstd*(r@W) - mu*rstd*(1@W) + b_eff,
so r is consumed in both orientations (r for bn_stats, r^T as matmul lhsT)
without any transposes of the normalized tensor.

All matmul operands are bf16 (1 PE cycle/row at any free size, half the
HBM bytes); PSUM accumulation and the residual path stay fp32.
Host-side precision sim: rel_err ~1.5e-3 vs the 2e-2 gate.
"""
import sys
if '/opt/trn_rl_repo' not in sys.path:
    sys.path.insert(0, '/opt/trn_rl_repo')
import math
import numpy as np
import ml_dtypes
import concourse.bass as bass
import concourse.bacc as bacc
import concourse.mybir as mybir
import concourse.tile as tile
from concourse.bass_utils import run_bass_kernel_spmd

AF = mybir.ActivationFunctionType
ALU = mybir.AluOpType
F32 = mybir.dt.float32
BF = mybir.dt.bfloat16
F8 = mybir.dt.float8e4

B, L, D, K = 1, 2048, 512, 64
NCORES = 8
R = L // NCORES          # 256 rows per core
NB = R // 128            # 2 l-blocks
ND = D // 128            # 4 d-tiles

RUN_KWARGS = {}          # test harness can inject trace=True etc.
LAST_RESULTS = None
_PROGRAM_CACHE = {}
DEBUG_DUMP = False
WARMUP_COLLECTIVE = False


def _build_program():
    nc = bacc.Bacc("TRN2", target_bir_lowering=False, debug=False,
                   num_devices=NCORES)

    din = {}
    def inp(name, shape, dt=BF):
        din[name] = nc.dram_tensor(name, list(shape), dt, kind="ExternalInput")
        return din[name]

    wk_d = inp("wk", [D, 768])                  # kw1 | w2k | wam
    wv_d = inp("wv", [D, D])                    # v_w
    wq_d = inp("wq", [D, 640])                  # qw1 | w2q
    wo_d = inp("wo", [D, D])                    # W_eff
    xTp_d = inp("xTp", [128, ND * R])
    cpackf_d = inp("cpackf", [128, 16], F32)
    cpackb_d = inp("cpackb", [128, 128])        # identity (bf16)
    wdiag_d = inp("wdiag8", [128, 896], F8)     # per-core prefix 0/1 diag x7
    maskf_d = inp("maskf", [128, 128], F32)
    miscv_d = inp("miscv", [1, D])              # vb
    w1b_d = inp("w1b", [128, D])                # -(1@W_eff) broadcast 128 rows
    x_d = inp("x_rm", [R, D], F32)              # x + b_eff (residual, fp32)
    y_d = nc.dram_tensor("y", [R, D], F32, kind="ExternalOutput")
    if DEBUG_DUMP:
        dbg_r_d = nc.dram_tensor("dbg_r", [R, D], F32, kind="ExternalOutput")
        dbg_rw_d = nc.dram_tensor("dbg_rw", [R, D], F32, kind="ExternalOutput")

    with tile.TileContext(nc) as tc:
        with tc.tile_pool(name="sb", bufs=1) as sbp, \
             tc.tile_pool(name="ps", bufs=1, space="PSUM") as ps, \
             tc.tile_pool(name="dr", bufs=1, space="DRAM") as dr:

            # ---------- small SBUF constants ----------
            ones1 = sbp.tile([1, 128], BF, name="ones1")
            nc.gpsimd.memset(ones1[:], 1.0)
            if WARMUP_COLLECTIVE:
                # tiny AllGather at t=0: pre-syncs the cores and pays the
                # collective channel-setup cost during the front DMA/compute
                wu_sb = sbp.tile([1, 16], BF, name="wu_sb")
                nc.gpsimd.memset(wu_sb[:], 0.0)
                wu_in = dr.tile([1, 16], BF, name="wu_in")
                wu_out = dr.tile([NCORES, 1, 16], BF, addr_space="Shared",
                                 name="wu_out")
                nc.sync.dma_start(wu_in[:], wu_sb[:])
                nc.gpsimd.collective_compute(
                    "AllGather", ALU.bypass,
                    replica_groups=[list(range(NCORES))],
                    ins=[wu_in[:]], outs=[wu_out[:]],
                )

            # ---------- front DMA (SP ring, k-path weights first) ----------
            def load_rows(name, dram, p, cols, n):
                ts = []
                for t in range(n):
                    tl = sbp.tile([p, cols], BF, name=f"{name}{t}")
                    nc.sync.dma_start(tl[:], dram[t * p:(t + 1) * p, :])
                    ts.append(tl)
                return ts

            cpackf = sbp.tile([128, 16], F32, name="cpackf")
            nc.sync.dma_start(cpackf[:], cpackf_d[:])
            xTp = sbp.tile([128, ND * R], BF, name="xTp")
            nc.sync.dma_start(xTp[:], xTp_d[:])
            wk = load_rows("wk", wk_d, 128, 768, ND)
            cpackb = sbp.tile([128, 128], BF, name="cpackb")
            nc.sync.dma_start(cpackb[:], cpackb_d[:])
            wv = load_rows("wv", wv_d, 128, D, ND)
            maskf = sbp.tile([128, 128], F32, name="maskf")
            nc.sync.dma_start(maskf[:], maskf_d[:])
            miscv = sbp.tile([1, D], BF, name="miscv")
            nc.sync.dma_start(miscv[:], miscv_d[:])
            wdiag = sbp.tile([128, 896], F8, name="wdiag")
            nc.sync.dma_start(wdiag[:], wdiag_d[:])
            w1b = sbp.tile([128, D], BF, name="w1b")
            nc.sync.dma_start(w1b[:], w1b_d[:])
            wq = load_rows("wq", wq_d, 128, 640, ND)
            wo = load_rows("wo", wo_d, 128, D, ND)

            sinsc, sinbs = cpackf[:, 0:1], cpackf[:, 1:2]
            epscol = [cpackf[:, 2:3], cpackf[:, 3:4]]
            b1k = [cpackf[:, 4 + i:5 + i] for i in range(ND)]
            b1q = [cpackf[:, 8 + i:9 + i] for i in range(ND)]
            b2k, b2q, bam = cpackf[:, 12:13], cpackf[:, 13:14], cpackf[:, 14:15]
            identb = cpackb[:]

            def xT(dj, lo=0, sz=R):
                return xTp[:, dj * R + lo: dj * R + lo + sz]

            # ---------- k path ----------
            hk = []
            for do in range(ND):
                hps = ps.tile([128, R], F32, name=f"hk_ps{do}", tag="pa", bufs=2)
                for dj in range(ND):
                    nc.tensor.matmul(hps[:], wk[dj][:, do*128:(do+1)*128],
                                     xT(dj), start=(dj == 0), stop=(dj == ND - 1))
                h = sbp.tile([128, R], BF, name=f"hk{do}")
                nc.scalar.activation(h[:], hps[:], AF.Gelu, bias=b1k[do])
                hk.append(h)
            pkps = ps.tile([128, R], F32, name="pk_ps", tag="pa", bufs=2)
            for do in range(ND):
                nc.tensor.matmul(pkps[:], wk[do][:, 512:640], hk[do][:],
                                 start=(do == 0), stop=(do == ND - 1))
            tk = sbp.tile([128, R], F32, name="tk")
            nc.scalar.activation(tk[:], pkps[:], AF.Tanh, bias=b2k)
            nc.scalar.activation(tk[0:64, :], tk[0:64, :], AF.Abs)
            csk = sbp.tile([128, R], F32, name="csk")
            nc.scalar.activation(csk[:], tk[:], AF.Sin, bias=sinbs, scale=sinsc)

            # ---------- amp (softplus via Exp+Ln, one table set) ----------
            amps = ps.tile([128, R], F32, name="am_ps", tag="pa", bufs=2)
            for dj in range(ND):
                nc.tensor.matmul(amps[:], wk[dj][:, 640:768], xT(dj),
                                 start=(dj == 0), stop=(dj == ND - 1))
            e_sb = sbp.tile([128, R], F32, name="e_sb")
            nc.scalar.activation(e_sb[:], amps[:], AF.Exp, bias=bam)
            al_sb = sbp.tile([128, R], F32, name="al_sb")
            nc.scalar.activation(al_sb[:], e_sb[:], AF.Ln, bias=1.0)

            phik = sbp.tile([128, R], BF, name="phik")
            nc.vector.scalar_tensor_tensor(phik[:], al_sb[:], 0.1, csk[:],
                                           ALU.add, ALU.mult)

            # ---------- V ----------
            V = []
            for lb in range(NB):
                vps = ps.tile([128, D], F32, name=f"v_ps{lb}", tag="pv", bufs=3)
                for dj in range(ND):
                    nc.tensor.matmul(vps[:], xT(dj, lb * 128, 128), wv[dj][:],
                                     start=(dj == 0), stop=False)
                nc.tensor.matmul(vps[:], ones1[0:1, :], miscv[:],
                                 start=False, stop=True, skip_group_check=True)
                v = sbp.tile([128, D], BF, name=f"V{lb}")
                nc.vector.tensor_copy(v[:], vps[:])
                V.append(v)

            # ---------- chunk state S + AllGather ----------
            phik_rm = []
            for tb in range(NB):
                trp = ps.tile([128, R], F32, name=f"tr_ps{tb}", tag="pa", bufs=2)
                nc.tensor.matmul(trp[:, 0:128], phik[:, tb*128:(tb+1)*128],
                                 identb, start=True, stop=True)
                k_rm = sbp.tile([128, 128], BF, name=f"phik_rm{tb}")
                nc.vector.tensor_copy(k_rm[:], trp[:, 0:128])
                phik_rm.append(k_rm)
            sps = ps.tile([128, D], F32, name="s_ps", tag="pv", bufs=3)
            for tb in range(NB):
                nc.tensor.matmul(sps[:], phik_rm[tb][:], V[tb][:],
                                 start=(tb == 0), stop=(tb == NB - 1))
            s_sb = sbp.tile([128, D], F8, name="s_sb")
            nc.vector.tensor_copy(s_sb[:], sps[:])
            x_rm = []
            for lb in range(NB):
                xr = sbp.tile([128, D], F32, name=f"x_rm{lb}")
                nc.sync.dma_start(xr[:], x_d[lb*128:(lb+1)*128, :])
                x_rm.append(xr)
            cc_in = dr.tile([128, D], F8, name="cc_in")
            cc_out = dr.tile([NCORES, 128, D], F8, addr_space="Shared",
                             name="cc_out")
            nc.sync.dma_start(cc_in[:], s_sb[:])
            nc.gpsimd.collective_compute(
                "AllGather", ALU.bypass,
                replica_groups=[list(range(NCORES))],
                ins=[cc_in[:]], outs=[cc_out[:]],
            )

            # ---------- q path (fills the AllGather window) ----------
            hq = []
            for do in range(ND):
                hps = ps.tile([128, R], F32, name=f"hq_ps{do}", tag="pa", bufs=2)
                for dj in range(ND):
                    nc.tensor.matmul(hps[:], wq[dj][:, do*128:(do+1)*128],
                                     xT(dj), start=(dj == 0), stop=(dj == ND - 1))
                h = sbp.tile([128, R], BF, name=f"hq{do}")
                nc.scalar.activation(h[:], hps[:], AF.Gelu, bias=b1q[do])
                hq.append(h)
            pqps = ps.tile([128, R], F32, name="pq_ps", tag="pa", bufs=2)
            for do in range(ND):
                nc.tensor.matmul(pqps[:], wq[do][:, 512:640], hq[do][:],
                                 start=(do == 0), stop=(do == ND - 1))
            tq = sbp.tile([128, R], F32, name="tq")
            nc.scalar.activation(tq[:], pqps[:], AF.Tanh, bias=b2q)
            nc.scalar.activation(tq[0:64, :], tq[0:64, :], AF.Abs)
            csq = sbp.tile([128, R], F32, name="csq")
            nc.scalar.activation(csq[:], tq[:], AF.Sin, bias=sinbs, scale=sinsc)
            phiq = sbp.tile([128, R], BF, name="phiq")
            nc.vector.scalar_tensor_tensor(phiq[:], al_sb[:], 0.1, csq[:],
                                           ALU.add, ALU.mult)

            # ---------- intra-chunk scores ----------
            a0ps = ps.tile([128, R], F32, name="a0_ps", tag="pa", bufs=2)
            nc.tensor.matmul(a0ps[:], phik[:, 0:128], phiq[:], start=True, stop=True)
            a0 = sbp.tile([128, R], BF, name="a0")
            nc.vector.tensor_tensor(a0[:, 0:128], a0ps[:, 0:128], maskf[:], ALU.mult)
            nc.vector.tensor_copy(a0[:, 128:256], a0ps[:, 128:256])
            a1ps = ps.tile([128, R], F32, name="a1_ps", tag="pa", bufs=2)
            nc.tensor.matmul(a1ps[:, 0:128], phik[:, 128:256], phiq[:, 128:256],
                             start=True, stop=True)
            a1 = sbp.tile([128, 128], BF, name="a1")
            nc.vector.tensor_tensor(a1[:], a1ps[:, 0:128], maskf[:], ALU.mult)

            # ---------- r (l,d) and rT (d,l) intra parts ----------
            rps = []
            for lb in range(NB):
                rp = ps.tile([128, D], F32, name=f"r_ps{lb}", tag="pv", bufs=3)
                if lb == 0:
                    nc.tensor.matmul(rp[:], a0[:, 0:128], V[0][:],
                                     start=True, stop=False)
                else:
                    nc.tensor.matmul(rp[:], a0[:, 128:256], V[0][:],
                                     start=True, stop=False)
                    nc.tensor.matmul(rp[:], a1[:], V[1][:],
                                     start=False, stop=False)
                rps.append(rp)
            # One open accumulation group per PSUM bank at a time: each pair
            # tile runs dt-even's full group (intra now, inter later) before
            # dt-odd's group opens post-collective.
            rtps = []
            for pair in range(2):            # pair p holds dtiles 2p, 2p+1
                rt = ps.tile([128, D], F32, name=f"rt_ps{pair}", tag="prt", bufs=2)
                dt = pair * 2
                nc.tensor.matmul(rt[:, 0:256],
                                 V[0][:, dt*128:(dt+1)*128], a0[:],
                                 start=True, stop=False,
                                 skip_group_check=True)
                nc.tensor.matmul(rt[:, 128:256],
                                 V[1][:, dt*128:(dt+1)*128], a1[:],
                                 start=False, stop=False,
                                 skip_group_check=True)
                rtps.append(rt)

            # ---------- prefix state P from gathered S ----------
            pps = ps.tile([128, D], F32, name="p_ps", tag="pv", bufs=3)
            for j in range(NCORES - 1):
                sa = sbp.tile([128, D], F8, name=f"s_all{j}")
                nc.sync.dma_start(sa[:], cc_out[j])
                nc.tensor.matmul(pps[:], wdiag[:, j*128:(j+1)*128], sa[:],
                                 start=(j == 0), stop=(j == NCORES - 2))
            p_sb = sbp.tile([128, D], BF, name="p_sb")
            nc.scalar.copy(p_sb[:, 0:256], pps[:, 0:256])
            nc.scalar.copy(p_sb[:, 256:512], pps[:, 256:512])

            # ---------- inter-chunk terms ----------
            for lb in range(NB):
                nc.tensor.matmul(rps[lb][:], phiq[:, lb*128:(lb+1)*128], p_sb[:],
                                 start=False, stop=True, skip_group_check=True)
            for pair in range(2):
                dt0, dt1 = pair * 2, pair * 2 + 1
                # close dt-even's group (inter term), then dt-odd's full group
                nc.tensor.matmul(rtps[pair][:, 0:256],
                                 p_sb[:, dt0*128:(dt0+1)*128], phiq[:],
                                 start=False, stop=True, skip_group_check=True)
                nc.tensor.matmul(rtps[pair][:, 256:512],
                                 V[0][:, dt1*128:(dt1+1)*128], a0[:],
                                 start=True, stop=False, skip_group_check=True)
                nc.tensor.matmul(rtps[pair][:, 384:512],
                                 V[1][:, dt1*128:(dt1+1)*128], a1[:],
                                 start=False, stop=False, skip_group_check=True)
                nc.tensor.matmul(rtps[pair][:, 256:512],
                                 p_sb[:, dt1*128:(dt1+1)*128], phiq[:],
                                 start=False, stop=True, skip_group_check=True)

            # ---------- LN stats + folded output projection ----------
            rt_sb = []
            for pair in range(2):
                rs = sbp.tile([128, D], BF, name=f"rt_sb{pair}")
                if pair == 0:
                    nc.scalar.copy(rs[:], rtps[pair][:])
                else:
                    nc.vector.tensor_copy(rs[:], rtps[pair][:])
                rt_sb.append(rs)
            if DEBUG_DUMP:
                for lb in range(NB):
                    rf = sbp.tile([128, D], F32, name=f"dbg_r{lb}")
                    nc.scalar.copy(rf[:], rps[lb][:])
                    nc.sync.dma_start(dbg_r_d[lb*128:(lb+1)*128, :], rf[:])
            rstds, bmus = [], []
            for lb in range(NB):
                bn6 = sbp.tile([128, 6], F32, name=f"bn6_{lb}")
                nc.vector.bn_stats(bn6[:], rps[lb][:])
                bn2 = sbp.tile([128, 2], F32, name=f"bn2_{lb}")
                nc.vector.bn_aggr(bn2[:], bn6[:])
                # irstd = sqrt(var+eps), rstd = 1/irstd (DVE reciprocal)
                irstd = sbp.tile([128, 1], F32, name=f"irstd{lb}")
                nc.scalar.activation(irstd[:], bn2[:, 1:2], AF.Sqrt,
                                     bias=epscol[lb])
                rstd = sbp.tile([128, 1], F32, name=f"rstd{lb}")
                nc.vector.reciprocal(rstd[:], irstd[:])
                rstds.append(rstd)
                bmu = sbp.tile([128, 1], F32, name=f"bmu{lb}")
                nc.vector.tensor_tensor(bmu[:], bn2[:, 0:1], rstd[:], ALU.mult)
                bmus.append(bmu)

            for lb in range(NB):
                rw = ps.tile([128, D], F32, name=f"rw_ps{lb}", tag="pv", bufs=3)
                for dt in range(ND):
                    pair, half = dt // 2, dt % 2
                    nc.tensor.matmul(rw[:],
                                     rt_sb[pair][:, half*256 + lb*128: half*256 + (lb+1)*128],
                                     wo[dt][:],
                                     start=(dt == 0), stop=(dt == ND - 1))
                if DEBUG_DUMP:
                    rwf = sbp.tile([128, D], F32, name=f"dbg_rw{lb}")
                    nc.scalar.copy(rwf[:], rw[:])
                    nc.sync.dma_start(dbg_rw_d[lb*128:(lb+1)*128, :], rwf[:])
                # y = rstd*(r@W) + bmu*(-w1row) + (x + b_eff)
                y1 = sbp.tile([128, D], F32, name=f"y1_{lb}")
                nc.vector.scalar_tensor_tensor(y1[:], rw[:], rstds[lb][:],
                                               x_rm[lb][:], ALU.mult, ALU.add)
                y_sb = sbp.tile([128, D], F32, name=f"y{lb}")
                nc.vector.scalar_tensor_tensor(y_sb[:], w1b[:], bmus[lb][:],
                                               y1[:], ALU.mult, ALU.add)
                nc.sync.dma_start(y_d[lb*128:(lb+1)*128, :], y_sb[:])

    nc.compile()
    return nc


def _bf16(a):
    return np.ascontiguousarray(np.asarray(a, np.float32).astype(ml_dtypes.bfloat16))


def kernel(**inputs):
    global LAST_RESULTS
    if 'prog' not in _PROGRAM_CACHE:
        _PROGRAM_CACHE['prog'] = _build_program()
    nc = _PROGRAM_CACHE['prog']

    f = {k: np.asarray(v, np.float32) for k, v in inputs.items()}
    x = f['x'][0]                                   # (L, D)
    W_eff = f['ln_g'][:, None] * f['out_w']
    b_eff = f['ln_b'] @ f['out_w'] + f['out_b']
    w1row = W_eff.sum(axis=0)

    wkp = np.concatenate([f['ke_w1'],
                          np.concatenate([f['ke_w2'], f['ke_w2']], 1),
                          np.concatenate([f['amp_w'], f['amp_w']], 1)], 1)
    wqp = np.concatenate([f['qe_w1'],
                          np.concatenate([f['qe_w2'], f['qe_w2']], 1)], 1)

    miscv = f['v_b'][None, :]
    w1b = np.broadcast_to(-w1row[None, :], (128, D)).copy()

    maskf = (np.arange(128)[None, :] >= np.arange(128)[:, None]).astype(np.float32)

    shared = {
        "wk": _bf16(wkp),
        "wv": _bf16(f['v_w']),
        "wq": _bf16(wqp),
        "wo": _bf16(W_eff),
        "miscv": _bf16(miscv),
        "w1b": _bf16(w1b),
        "maskf": maskf,
        "cpackb": _bf16(np.eye(128, dtype=np.float32)),
    }

    in_maps = []
    for c in range(NCORES):
        xc = x[R*c:R*(c+1)]
        xTp = np.zeros((128, ND * R), np.float32)
        for t in range(ND):
            xTp[:, t*R:(t+1)*R] = xc[:, t*128:(t+1)*128].T
        cpackf = np.zeros((128, 16), np.float32)
        cpackf[0:64, 0] = -math.pi
        cpackf[64:128, 0] = math.pi
        cpackf[0:64, 1] = math.pi / 2
        for lb in range(NB):
            gl = np.arange(c*R + lb*128, c*R + (lb+1)*128, dtype=np.float64)
            cpackf[:, 2 + lb] = (1e-5 * K * (gl + 1)).astype(np.float32)
        for i in range(ND):
            cpackf[:, 4 + i] = f['ke_b1'][i*128:(i+1)*128]
            cpackf[:, 8 + i] = f['qe_b1'][i*128:(i+1)*128]
        cpackf[:, 12] = np.concatenate([f['ke_b2'], f['ke_b2']])
        cpackf[:, 13] = np.concatenate([f['qe_b2'], f['qe_b2']])
        cpackf[:, 14] = np.concatenate([f['amp_b'], f['amp_b']])
        wdiag = np.zeros((128, 896), np.float32)
        eye = np.eye(128, dtype=np.float32)
        for j in range(NCORES - 1):
            if j < c:
                wdiag[:, j*128:(j+1)*128] = eye
        in_maps.append({
            **shared,
            "xTp": _bf16(xTp),
            "cpackf": np.ascontiguousarray(cpackf),
            "wdiag8": np.ascontiguousarray(
                wdiag.astype(ml_dtypes.float8_e4m3)),
            "x_rm": np.ascontiguousarray(xc + b_eff[None, :].astype(np.float32)),
        })

    res = run_bass_kernel_spmd(nc, in_maps, core_ids=list(range(NCORES)),
                               **RUN_KWARGS)
    LAST_RESULTS = res
    y = np.concatenate([res.results[c]['y'] for c in range(NCORES)], axis=0)
    return y[None].astype(np.float32)
